# revision 23
# baseline (speedup 1.0000x reference)
"""Trainium2 Bass kernel for nn_MessageProp (gnn_message_passing).

Reference computation (B=65536 rows, D=128, K=8 components, H=132 hidden):
    msgs  = einsum('kbd,ed->kbe', components, Wm) + bm   # message_map per component
    right = msgs.sum(0) @ Wu.T + bu                      # update_map
    x     = concat([signal, right], -1)
    h0 = relu(x @ W0.T + b0); h1 = relu(h0 @ W1.T + b1); h2 = relu(h1 @ W2.T + b2)
    out = h2 @ W3.T + b3

Key algebraic folds done on the host (all linear maps commute with the k-sum):
    csum = sum_k components[k]
    pre0 = signal @ A.T + csum @ Cm.T + b0'
      A   = W0[:, :D]
      Cm  = W0[:, D:] @ Wu @ Wm
      b0' = b0 + W0[:, D:] @ (Wu @ (K*bm) + bu)
so the device only computes csum and a 4-matmul-layer MLP in feature-major
layout (PE transposes at tile boundaries).

bf16 on-chip pipeline: inputs are cast fp32->bf16 during the (SWDGE) load,
halving SBUF write traffic and doubling DVE merge throughput; all matmuls are
bf16 x bf16 with fp32 PSUM accumulation (and FWL fast weight loads); the final
output is converted back to fp32 on the PSUM->SBUF copy. Biases stay fp32.

Sharding: data-parallel over B across 8 cores (8192 rows each); weights
replicated.
"""

import numpy as np
import ml_dtypes
from contextlib import ExitStack, nullcontext

import concourse.bass as bass
import concourse.bacc as bacc
import concourse.tile as tile
import concourse.mybir as mybir
from concourse import bass_utils

F32 = mybir.dt.float32
BF16 = mybir.dt.bfloat16
ACT = mybir.ActivationFunctionType
ADD = mybir.AluOpType.add
MAX = mybir.AluOpType.max

D = 128          # latent dim
H = 132          # FCBlock hidden width
B = 65536        # batch
K = 8            # components
NCORES = 8
RB = B // NCORES  # 8192 rows per core
SUB = 4          # m-blocks (128 rows each) per compute sub-tile -> 512 rows

# tapered row-tile sizes (sum = RB); small final tiles shrink the drain tail
TILES = (1024,) * 7 + (512, 512)
# component-load gangs: list of (k0, k1) slices, one SWDGE cast-DMA each
GANGS = ((0, 8),)
# finer gangs for tile 0 only: the merge tree can start at the first
# quarter-gang, shortening the pipeline ramp
GANGS_T0 = ((0, 2), (2, 4), (4, 6), (6, 8))
BUFS_LOADS = 3
BUFS_ACTS = 3
BUFS_OUT = 3      # must be >= STORE_DELAY + 1
STORE_DELAY = 2   # issue store(t-DELAY) after loads(t) so its sem wait
                  # never head-of-line-blocks later tiles' loads
REPS = 1          # timing harness: repeat body via HW loop
SKIP_COMPUTE = False  # timing-only: loads+merge+store, no MLP
PS_IN_SHARED = False  # sig+cs transposes share one psum bank per subtile
# PSUM bank budget (8 total)
B_IN = 2
B_HA = 3
B_HB = 1
B_PO = 1
B_PO2 = 1

# bf16 weight-pack column layout [128, NWH]
_H_IDENT = 0
_H_W0A_SIG = 128
_H_W0A_CS = 256
_H_W1A_HI = 384
_H_W2A_HI = 512
_H_W3_HI = 640
_H_W1A_LO = 768    # [4,128] on partitions 0:4
_H_W2A_LO = 896
_H_W3_LO = 1024
_H_W0B_SIG = 1152  # [128,4]
_H_W0B_CS = 1156
_H_W1B_HI = 1160
_H_W2B_HI = 1164
_H_W1B_LO = 1168   # [4,4] on partitions 0:4
_H_W2B_LO = 1172
NWH = 1176

# fp32 bias-pack column layout [128, NWF]
_F_B0A = 0
_F_B1A = 1
_F_B2A = 2
_F_B3 = 3
_F_B0B = 4         # [4,1] on partitions 0:4
_F_B1B = 5
_F_B2B = 6
NWF = 7


def _build_wpacks(Wm, bm, Wu, bu, W0, b0, W1, b1, W2, b2, W3, b3):
    f8 = np.float64
    Wm, bm, Wu, bu = Wm.astype(f8), bm.astype(f8), Wu.astype(f8), bu.astype(f8)
    W0, b0, W1, b1 = W0.astype(f8), b0.astype(f8), W1.astype(f8), b1.astype(f8)
    W2, b2, W3, b3 = W2.astype(f8), b2.astype(f8), W3.astype(f8), b3.astype(f8)

    A = W0[:, :D]                              # [H, D]
    W0r = W0[:, D:]                            # [H, D]
    Cm = W0r @ (Wu @ Wm)                       # [H, D]
    b0p = b0 + W0r @ (Wu @ (K * bm) + bu)      # [H]

    wh = np.zeros((128, NWH), dtype=np.float64)
    wh[:, _H_IDENT:_H_IDENT + 128] = np.eye(128)
    # L0: lhsT[p=d, m=h] = A.T / Cm.T
    wh[:, _H_W0A_SIG:_H_W0A_SIG + 128] = A.T[:, :128]
    wh[:, _H_W0A_CS:_H_W0A_CS + 128] = Cm.T[:, :128]
    wh[:, _H_W0B_SIG:_H_W0B_SIG + 4] = A.T[:, 128:]
    wh[:, _H_W0B_CS:_H_W0B_CS + 4] = Cm.T[:, 128:]
    # L1/L2: lhsT[p=h_in, m=h_out] = Wx.T
    for Wx, chi, clo, cbhi, cblo in (
        (W1, _H_W1A_HI, _H_W1A_LO, _H_W1B_HI, _H_W1B_LO),
        (W2, _H_W2A_HI, _H_W2A_LO, _H_W2B_HI, _H_W2B_LO),
    ):
        WT = Wx.T                              # [132 in, 132 out]
        wh[:, chi:chi + 128] = WT[:128, :128]
        wh[:4, clo:clo + 128] = WT[128:, :128]
        wh[:, cbhi:cbhi + 4] = WT[:128, 128:]
        wh[:4, cblo:cblo + 4] = WT[128:, 128:]
    # L3: lhsT[p=h2, m=d] = W3.T
    W3T = W3.T                                 # [132, 128]
    wh[:, _H_W3_HI:_H_W3_HI + 128] = W3T[:128, :]
    wh[:4, _H_W3_LO:_H_W3_LO + 128] = W3T[128:, :]

    wf = np.zeros((128, NWF), dtype=np.float64)
    wf[:, _F_B0A] = b0p[:128]
    wf[:, _F_B1A] = b1[:128]
    wf[:, _F_B2A] = b2[:128]
    wf[:, _F_B3] = b3
    wf[:4, _F_B0B] = b0p[128:]
    wf[:4, _F_B1B] = b1[128:]
    wf[:4, _F_B2B] = b2[128:]
    return (np.ascontiguousarray(wh.astype(ml_dtypes.bfloat16)),
            np.ascontiguousarray(wf.astype(np.float32)))


def _trace_kernel(nc: bass.Bass):
    assert sum(TILES) == RB and all(tl % (SUB * 128) == 0 for tl in TILES)
    sig = nc.dram_tensor("sig", [RB, D], F32, kind="ExternalInput")
    comp = nc.dram_tensor("comp", [K, RB, D], F32, kind="ExternalInput")
    wpackh = nc.dram_tensor("wpackh", [128, NWH], BF16, kind="ExternalInput")
    wpackf = nc.dram_tensor("wpackf", [128, NWF], F32, kind="ExternalInput")
    out = nc.dram_tensor("out", [RB, D], F32, kind="ExternalOutput")

    # per-tile views; within tile t: row = r0 + p*M_t + m, free layout (m d)
    def tile_views(r0, tl, gangs):
        m = tl // 128
        s_v = sig.ap()[r0:r0 + tl, :].rearrange("(p m) d -> p (m d)", p=128, m=m)
        g_v = [comp.ap()[k0:k1, r0:r0 + tl, :]
               .rearrange("k (p m) d -> p k (m d)", p=128, m=m)
               for k0, k1 in gangs]
        o_v = out.ap()[r0:r0 + tl, :].rearrange("(p m) d -> p (m d)", p=128, m=m)
        return s_v, g_v, o_v

    with tile.TileContext(nc) as tc, ExitStack() as ctx:
        wpool = ctx.enter_context(tc.tile_pool(name="weights", bufs=1))
        loads = ctx.enter_context(tc.tile_pool(name="loads", bufs=BUFS_LOADS))
        acts = ctx.enter_context(tc.tile_pool(name="acts", bufs=BUFS_ACTS))
        opool = ctx.enter_context(tc.tile_pool(name="outs", bufs=BUFS_OUT))
        psum = ctx.enter_context(tc.tile_pool(name="psum", bufs=2, space="PSUM"))

        wh_sb = wpool.tile([128, NWH], BF16)
        nc.sync.dma_start(wh_sb[:], wpackh.ap())
        wf_sb = wpool.tile([128, NWF], F32)
        nc.sync.dma_start(wf_sb[:], wpackf.ap())

        ident = wh_sb[:, _H_IDENT:_H_IDENT + 128]

        def wh(c, n=128, parts=128):
            return wh_sb[:parts, c:c + n]

        def wf(c, parts=128):
            return wf_sb[:parts, c:c + 1]

        with (tc.For_i(0, REPS, 1) if REPS > 1 else nullcontext()):
            r0 = 0
            pend_stores = []

            def flush_store():
                o_v, o_sb = pend_stores.pop(0)
                nc.sync.dma_start(o_v, o_sb[:])

            for t, TLt in enumerate(TILES):
                NSUB = TLt // (SUB * 128)
                gangs = GANGS_T0 if (t == 0 and GANGS_T0) else GANGS
                sig_v, gang_v, out_v = tile_views(r0, TLt, gangs)
                r0 += TLt

                sig_h = loads.tile([128, TLt], BF16, tag="sig_h")
                nc.gpsimd.dma_start(sig_h[:], sig_v)
                comp_h = loads.tile([128, K * TLt], BF16, tag="comp_h")
                for (k0, k1), g_v in zip(gangs, gang_v):
                    nc.gpsimd.dma_start(comp_h[:, k0 * TLt:k1 * TLt], g_v)

                # merge tree on DVE (bf16, 2x lane rate)
                ck = lambda k: comp_h[:, k * TLt:(k + 1) * TLt]
                ta = loads.tile([128, TLt], BF16, tag="ta")
                tb = loads.tile([128, TLt], BF16, tag="tb")
                cs_h = loads.tile([128, TLt], BF16, tag="cs_h")
                nc.vector.tensor_add(ta[:], ck(0), ck(1))
                nc.vector.tensor_add(tb[:], ck(4), ck(5))
                nc.vector.tensor_add(ta[:], ta[:], ck(2))
                nc.vector.tensor_add(tb[:], tb[:], ck(6))
                nc.vector.tensor_add(ta[:], ta[:], ck(3))
                nc.vector.tensor_add(tb[:], tb[:], ck(7))
                nc.vector.tensor_add(cs_h[:], ta[:], tb[:])

                while len(pend_stores) >= STORE_DELAY:
                    flush_store()

                out_sb = opool.tile([128, TLt], F32, tag="out_sb")

                if SKIP_COMPUTE:
                    nc.vector.tensor_copy(out_sb[:], sig_h[:])
                    pend_stores.append((out_v, out_sb))
                    continue

                for s in range(NSUB):
                    cols = slice(s * SUB * 128, (s + 1) * SUB * 128)

                    # ---- transpose signal + csum blocks into feature-major ----
                    if PS_IN_SHARED:
                        # both into ONE bf16 psum bank so B_IN=2
                        # double-buffers across subtiles
                        ps_in = psum.tile([128, 2 * SUB * 128], BF16,
                                          tag="ps_in", bufs=B_IN)
                        ps_sig = ps_in[:, :SUB * 128]
                        ps_cs = ps_in[:, SUB * 128:]
                        for j in range(SUB):
                            mb = (s * SUB + j) * 128
                            nc.tensor.transpose(
                                ps_sig[:, j * 128:(j + 1) * 128],
                                sig_h[:, mb:mb + 128], ident)
                            nc.tensor.transpose(
                                ps_cs[:, j * 128:(j + 1) * 128],
                                cs_h[:, mb:mb + 128], ident)
                    else:
                        ps_sig = psum.tile([128, SUB * 128], BF16,
                                           tag="ps_in", bufs=B_IN)
                        for j in range(SUB):
                            mb = (s * SUB + j) * 128
                            nc.tensor.transpose(
                                ps_sig[:, j * 128:(j + 1) * 128],
                                sig_h[:, mb:mb + 128], ident)
                        ps_cs = psum.tile([128, SUB * 128], BF16,
                                          tag="ps_in", bufs=B_IN)
                        for j in range(SUB):
                            mb = (s * SUB + j) * 128
                            nc.tensor.transpose(
                                ps_cs[:, j * 128:(j + 1) * 128],
                                cs_h[:, mb:mb + 128], ident)
                    sigT = acts.tile([128, SUB * 128], BF16, tag="sigT")
                    nc.scalar.activation(sigT[:], ps_sig[:], ACT.Copy)
                    csT = acts.tile([128, SUB * 128], BF16, tag="csT")
                    nc.vector.tensor_copy(csT[:], ps_cs[:])

                    # ---- L0: h0 = relu(A@sigT + Cm@csT + b0') ----
                    ps_h0a = psum.tile([128, SUB * 128], F32, tag="ha",
                                       bufs=B_HA)
                    nc.tensor.matmul(ps_h0a[:], wh(_H_W0A_SIG),
                                     sigT[:], start=True, stop=False)
                    nc.tensor.matmul(ps_h0a[:], wh(_H_W0A_CS),
                                     csT[:], start=False, stop=True)
                    ps_h0b = psum.tile([4, SUB * 128], F32, tag="hb", bufs=B_HB)
                    nc.tensor.matmul(ps_h0b[:], wh(_H_W0B_SIG, 4),
                                     sigT[:], start=True, stop=False)
                    nc.tensor.matmul(ps_h0b[:], wh(_H_W0B_CS, 4),
                                     csT[:], start=False, stop=True)
                    h0a = acts.tile([128, SUB * 128], BF16, tag="h0a")
                    nc.vector.tensor_scalar(h0a[:], ps_h0a[:],
                                            wf(_F_B0A), 0.0, ADD, MAX)
                    h0b = acts.tile([4, SUB * 128], BF16, tag="h0b")
                    nc.scalar.activation(h0b[:], ps_h0b[:], ACT.Relu,
                                         bias=wf(_F_B0B, parts=4))

                    # ---- L1 ----
                    ps_h1a = psum.tile([128, SUB * 128], F32, tag="ha",
                                       bufs=B_HA)
                    nc.tensor.matmul(ps_h1a[:], wh(_H_W1A_HI),
                                     h0a[:], start=True, stop=False)
                    nc.tensor.matmul(ps_h1a[:], wh(_H_W1A_LO, 128, parts=4),
                                     h0b[:], start=False, stop=True)
                    ps_h1b = psum.tile([4, SUB * 128], F32, tag="hb", bufs=B_HB)
                    nc.tensor.matmul(ps_h1b[:], wh(_H_W1B_HI, 4),
                                     h0a[:], start=True, stop=False)
                    nc.tensor.matmul(ps_h1b[:], wh(_H_W1B_LO, 4, parts=4),
                                     h0b[:], start=False, stop=True)
                    h1a = acts.tile([128, SUB * 128], BF16, tag="h1a")
                    nc.vector.tensor_scalar(h1a[:], ps_h1a[:],
                                            wf(_F_B1A), 0.0, ADD, MAX)
                    h1b = acts.tile([4, SUB * 128], BF16, tag="h1b")
                    nc.scalar.activation(h1b[:], ps_h1b[:], ACT.Relu,
                                         bias=wf(_F_B1B, parts=4))

                    # ---- L2 ----
                    ps_h2a = psum.tile([128, SUB * 128], F32, tag="ha",
                                       bufs=B_HA)
                    nc.tensor.matmul(ps_h2a[:], wh(_H_W2A_HI),
                                     h1a[:], start=True, stop=False)
                    nc.tensor.matmul(ps_h2a[:], wh(_H_W2A_LO, 128, parts=4),
                                     h1b[:], start=False, stop=True)
                    ps_h2b = psum.tile([4, SUB * 128], F32, tag="hb", bufs=B_HB)
                    nc.tensor.matmul(ps_h2b[:], wh(_H_W2B_HI, 4),
                                     h1a[:], start=True, stop=False)
                    nc.tensor.matmul(ps_h2b[:], wh(_H_W2B_LO, 4, parts=4),
                                     h1b[:], start=False, stop=True)
                    h2a = acts.tile([128, SUB * 128], BF16, tag="h2a")
                    nc.scalar.activation(h2a[:], ps_h2a[:], ACT.Relu,
                                         bias=wf(_F_B2A))
                    h2b = acts.tile([4, SUB * 128], BF16, tag="h2b")
                    nc.scalar.activation(h2b[:], ps_h2b[:], ACT.Relu,
                                         bias=wf(_F_B2B, parts=4))

                    # ---- L3: outT = W3 @ h2 + b3 (feature-major) ----
                    ps_oT = psum.tile([128, SUB * 128], F32, tag="po",
                                      bufs=B_PO)
                    nc.tensor.matmul(ps_oT[:], wh(_H_W3_HI),
                                     h2a[:], start=True, stop=False)
                    nc.tensor.matmul(ps_oT[:], wh(_H_W3_LO, 128, parts=4),
                                     h2b[:], start=False, stop=True)
                    oT = acts.tile([128, SUB * 128], BF16, tag="oT")
                    nc.scalar.activation(oT[:], ps_oT[:], ACT.Identity,
                                         bias=wf(_F_B3))

                    # ---- transpose back to row-major and stage the store ----
                    ps_on = psum.tile([128, SUB * 128], BF16, tag="po2",
                                      bufs=B_PO2)
                    for j in range(SUB):
                        nc.tensor.transpose(ps_on[:, j * 128:(j + 1) * 128],
                                            oT[:, j * 128:(j + 1) * 128], ident)
                    nc.vector.tensor_copy(out_sb[:, cols], ps_on[:])

                pend_stores.append((out_v, out_sb))

            while pend_stores:
                flush_store()

    return nc


_CACHED_NC = None


def _get_nc():
    global _CACHED_NC
    if _CACHED_NC is None:
        nc = bacc.Bacc("TRN2", target_bir_lowering=False, debug=False,
                       enable_asserts=False, num_devices=NCORES)
        _trace_kernel(nc)
        nc.compile()
        _CACHED_NC = nc
    return _CACHED_NC


def _make_in_maps(inputs):
    signal = np.ascontiguousarray(np.asarray(inputs["signal"], np.float32))
    components = np.ascontiguousarray(np.asarray(inputs["components"],
                                                 np.float32))
    wh, wfp = _build_wpacks(*[np.asarray(inputs[k], np.float32) for k in
                              ("Wm", "bm", "Wu", "bu", "W0", "b0",
                               "W1", "b1", "W2", "b2", "W3", "b3")])
    in_maps = []
    for c in range(NCORES):
        r0 = c * RB
        in_maps.append({
            "sig": signal[r0:r0 + RB],
            "comp": np.ascontiguousarray(components[:, r0:r0 + RB, :]),
            "wpackh": wh,
            "wpackf": wfp,
        })
    return in_maps


def kernel(**inputs):
    nc = _get_nc()
    res = bass_utils.run_bass_kernel_spmd(nc, _make_in_maps(inputs),
                                          core_ids=list(range(NCORES)))
    return np.concatenate([res.results[c]["out"] for c in range(NCORES)],
                          axis=0)


# revision 32
# speedup vs baseline: 1.1146x; 1.1146x over previous
"""Trainium2 Bass kernel for nn_MessageProp (gnn_message_passing).

Reference computation (B=65536 rows, D=128, K=8 components, H=132 hidden):
    msgs  = einsum('kbd,ed->kbe', components, Wm) + bm   # message_map per component
    right = msgs.sum(0) @ Wu.T + bu                      # update_map
    x     = concat([signal, right], -1)
    h0 = relu(x @ W0.T + b0); h1 = relu(h0 @ W1.T + b1); h2 = relu(h1 @ W2.T + b2)
    out = h2 @ W3.T + b3

Key algebraic folds done on the host (all linear maps commute with the k-sum):
    csum = sum_k components[k]
    pre0 = signal @ A.T + csum @ Cm.T + b0'
      A   = W0[:, :D]
      Cm  = W0[:, D:] @ Wu @ Wm
      b0' = b0 + W0[:, D:] @ (Wu @ (K*bm) + bu)
so the device only computes csum and a 4-matmul-layer MLP in feature-major
layout (PE transposes at tile boundaries).

bf16 on-chip pipeline: inputs are cast fp32->bf16 during the (SWDGE) load,
halving SBUF write traffic and doubling DVE merge throughput; all matmuls are
bf16 x bf16 with fp32 PSUM accumulation (and FWL fast weight loads); the final
output is converted back to fp32 on the PSUM->SBUF copy. Biases stay fp32.

Sharding: data-parallel over B across 8 cores (8192 rows each); weights
replicated.
"""

import numpy as np
import ml_dtypes
from contextlib import ExitStack, nullcontext

import concourse.bass as bass
import concourse.bacc as bacc
import concourse.tile as tile
import concourse.mybir as mybir
from concourse import bass_utils

F32 = mybir.dt.float32
BF16 = mybir.dt.bfloat16
ACT = mybir.ActivationFunctionType
ADD = mybir.AluOpType.add
MAX = mybir.AluOpType.max

D = 128          # latent dim
H = 132          # FCBlock hidden width
B = 65536        # batch
K = 8            # components
NCORES = 8
RB = B // NCORES  # 8192 rows per core
SUB = 4          # m-blocks (128 rows each) per compute sub-tile -> 512 rows

# tapered row-tile sizes (sum = RB); small final tiles shrink the drain tail
TILES = (1024,) * 7 + (512, 512)
# component-load gangs: list of (k0, k1) slices, one SWDGE cast-DMA each
GANGS = ((0, 8),)
# tile 0 only: split the gang DMA by column halves. HW-measured WORSE
# (+10us: strided half-gang descriptors cost more than the ramp gain);
# keep disabled
COL_SPLIT_T0 = None
BUFS_LOADS = 3
BUFS_ACTS = 3
BUFS_OUT = 3      # must be >= STORE_DELAY + 1
STORE_DELAY = 2   # issue store(t-DELAY) after loads(t) so its sem wait
                  # never head-of-line-blocks later tiles' loads
REPS = 1          # timing harness: repeat body via HW loop
SKIP_COMPUTE = False  # timing-only: loads+merge+store, no MLP
PS_IN_SHARED = False  # sig+cs transposes share one psum bank per subtile
MERGE_BALANCED = True  # depth-3 balanced merge tree (vs depth-4 chains)
PE_WARM = 40           # dummy PE transposes at start (clock-gate warmup)
# PSUM bank budget (8 total)
B_IN = 2
B_HA = 3
B_HB = 1
B_PO = 1
B_PO2 = 1

# bf16 weight-pack column layout [128, NWH]
_H_IDENT = 0
_H_W0A_SIG = 128
_H_W0A_CS = 256
_H_W1A_HI = 384
_H_W2A_HI = 512
_H_W3_HI = 640
_H_W1A_LO = 768    # [4,128] on partitions 0:4
_H_W2A_LO = 896
_H_W3_LO = 1024
_H_W0B_SIG = 1152  # [128,4]
_H_W0B_CS = 1156
_H_W1B_HI = 1160
_H_W2B_HI = 1164
_H_W1B_LO = 1168   # [4,4] on partitions 0:4
_H_W2B_LO = 1172
NWH = 1176

# fp32 bias-pack column layout [128, NWF]
_F_B0A = 0
_F_B1A = 1
_F_B2A = 2
_F_B3 = 3
_F_B0B = 4         # [4,1] on partitions 0:4
_F_B1B = 5
_F_B2B = 6
NWF = 7


def _build_wpacks(Wm, bm, Wu, bu, W0, b0, W1, b1, W2, b2, W3, b3):
    f8 = np.float64
    Wm, bm, Wu, bu = Wm.astype(f8), bm.astype(f8), Wu.astype(f8), bu.astype(f8)
    W0, b0, W1, b1 = W0.astype(f8), b0.astype(f8), W1.astype(f8), b1.astype(f8)
    W2, b2, W3, b3 = W2.astype(f8), b2.astype(f8), W3.astype(f8), b3.astype(f8)

    A = W0[:, :D]                              # [H, D]
    W0r = W0[:, D:]                            # [H, D]
    Cm = W0r @ (Wu @ Wm)                       # [H, D]
    b0p = b0 + W0r @ (Wu @ (K * bm) + bu)      # [H]

    wh = np.zeros((128, NWH), dtype=np.float64)
    wh[:, _H_IDENT:_H_IDENT + 128] = np.eye(128)
    # L0: lhsT[p=d, m=h] = A.T / Cm.T
    wh[:, _H_W0A_SIG:_H_W0A_SIG + 128] = A.T[:, :128]
    wh[:, _H_W0A_CS:_H_W0A_CS + 128] = Cm.T[:, :128]
    wh[:, _H_W0B_SIG:_H_W0B_SIG + 4] = A.T[:, 128:]
    wh[:, _H_W0B_CS:_H_W0B_CS + 4] = Cm.T[:, 128:]
    # L1/L2: lhsT[p=h_in, m=h_out] = Wx.T
    for Wx, chi, clo, cbhi, cblo in (
        (W1, _H_W1A_HI, _H_W1A_LO, _H_W1B_HI, _H_W1B_LO),
        (W2, _H_W2A_HI, _H_W2A_LO, _H_W2B_HI, _H_W2B_LO),
    ):
        WT = Wx.T                              # [132 in, 132 out]
        wh[:, chi:chi + 128] = WT[:128, :128]
        wh[:4, clo:clo + 128] = WT[128:, :128]
        wh[:, cbhi:cbhi + 4] = WT[:128, 128:]
        wh[:4, cblo:cblo + 4] = WT[128:, 128:]
    # L3: lhsT[p=h2, m=d] = W3.T
    W3T = W3.T                                 # [132, 128]
    wh[:, _H_W3_HI:_H_W3_HI + 128] = W3T[:128, :]
    wh[:4, _H_W3_LO:_H_W3_LO + 128] = W3T[128:, :]

    wf = np.zeros((128, NWF), dtype=np.float64)
    wf[:, _F_B0A] = b0p[:128]
    wf[:, _F_B1A] = b1[:128]
    wf[:, _F_B2A] = b2[:128]
    wf[:, _F_B3] = b3
    wf[:4, _F_B0B] = b0p[128:]
    wf[:4, _F_B1B] = b1[128:]
    wf[:4, _F_B2B] = b2[128:]
    return (np.ascontiguousarray(wh.astype(ml_dtypes.bfloat16)),
            np.ascontiguousarray(wf.astype(np.float32)))


def _trace_kernel(nc: bass.Bass):
    assert sum(TILES) == RB and all(tl % (SUB * 128) == 0 for tl in TILES)
    sig = nc.dram_tensor("sig", [RB, D], F32, kind="ExternalInput")
    comp = nc.dram_tensor("comp", [K, RB, D], F32, kind="ExternalInput")
    wpackh = nc.dram_tensor("wpackh", [128, NWH], BF16, kind="ExternalInput")
    wpackf = nc.dram_tensor("wpackf", [128, NWF], F32, kind="ExternalInput")
    out = nc.dram_tensor("out", [RB, D], F32, kind="ExternalOutput")

    # per-tile views; within tile t: row = r0 + p*M_t + m, free layout (m d)
    def tile_views(r0, tl, gangs):
        m = tl // 128
        s_v = sig.ap()[r0:r0 + tl, :].rearrange("(p m) d -> p (m d)", p=128, m=m)
        g_v = [comp.ap()[k0:k1, r0:r0 + tl, :]
               .rearrange("k (p m) d -> p k (m d)", p=128, m=m)
               for k0, k1 in gangs]
        o_v = out.ap()[r0:r0 + tl, :].rearrange("(p m) d -> p (m d)", p=128, m=m)
        return s_v, g_v, o_v

    with tile.TileContext(nc) as tc, ExitStack() as ctx:
        wpool = ctx.enter_context(tc.tile_pool(name="weights", bufs=1))
        loads = ctx.enter_context(tc.tile_pool(name="loads", bufs=BUFS_LOADS))
        acts = ctx.enter_context(tc.tile_pool(name="acts", bufs=BUFS_ACTS))
        opool = ctx.enter_context(tc.tile_pool(name="outs", bufs=BUFS_OUT))
        psum = ctx.enter_context(tc.tile_pool(name="psum", bufs=2, space="PSUM"))

        wh_sb = wpool.tile([128, NWH], BF16)
        nc.sync.dma_start(wh_sb[:], wpackh.ap())
        wf_sb = wpool.tile([128, NWF], F32)
        nc.sync.dma_start(wf_sb[:], wpackf.ap())

        ident = wh_sb[:, _H_IDENT:_H_IDENT + 128]

        def wh(c, n=128, parts=128):
            return wh_sb[:parts, c:c + n]

        def wf(c, parts=128):
            return wf_sb[:parts, c:c + 1]

        # PE clock-gating pre-warm: ~40 dummy transposes (~4.5us sustained)
        # during the tile-0 load window, so real matmuls start at the boosted
        # clock. Results land in the po2-tagged bank and are fully overwritten
        # (start=True per region) before any read.
        if PE_WARM:
            ps_warm = psum.tile([128, SUB * 128], BF16, tag="po2",
                                bufs=B_PO2)
            for _ in range(PE_WARM):
                nc.tensor.transpose(ps_warm[:, :128], ident, ident)

        with (tc.For_i(0, REPS, 1) if REPS > 1 else nullcontext()):
            r0 = 0
            pend_stores = []

            def flush_store():
                o_v, o_sb = pend_stores.pop(0)
                nc.sync.dma_start(o_v, o_sb[:])

            for t, TLt in enumerate(TILES):
                NSUB = TLt // (SUB * 128)
                sig_v, gang_v, out_v = tile_views(r0, TLt, GANGS)
                r0 += TLt

                sig_h = loads.tile([128, TLt], BF16, tag="sig_h")
                nc.gpsimd.dma_start(sig_h[:], sig_v)
                comp_h = loads.tile([128, K * TLt], BF16, tag="comp_h")
                splits = COL_SPLIT_T0 if (t == 0 and COL_SPLIT_T0) else 1
                for (k0, k1), g_v in zip(GANGS, gang_v):
                    if splits == 1:
                        nc.gpsimd.dma_start(comp_h[:, k0 * TLt:k1 * TLt], g_v)
                        continue
                    cw = TLt // splits
                    for h in range(splits):
                        # comp_h free layout is (k m d): the same column
                        # range of every k slice, strided
                        dst = comp_h[:, k0 * TLt:k1 * TLt] \
                            .rearrange("p (k c) -> p k c", k=k1 - k0)[
                                :, :, h * cw:(h + 1) * cw]
                        nc.gpsimd.dma_start(dst, g_v[:, :, h * cw:(h + 1) * cw])

                # merge tree on DVE, chunked per subtile so each subtile's
                # cs columns are ready as early as possible
                ta = loads.tile([128, TLt], BF16, tag="ta")
                tb = loads.tile([128, TLt], BF16, tag="tb")
                cs_h = loads.tile([128, TLt], BF16, tag="cs_h")
                if MERGE_BALANCED:
                    tc_ = loads.tile([128, TLt], BF16, tag="tc")
                    td = loads.tile([128, TLt], BF16, tag="td")
                CW = SUB * 128
                for c in range(TLt // CW):
                    cl = slice(c * CW, (c + 1) * CW)
                    ck = lambda k: comp_h[:, k * TLt + c * CW:
                                          k * TLt + (c + 1) * CW]
                    if MERGE_BALANCED:
                        nc.vector.tensor_add(ta[:, cl], ck(0), ck(1))
                        nc.vector.tensor_add(tb[:, cl], ck(2), ck(3))
                        nc.vector.tensor_add(tc_[:, cl], ck(4), ck(5))
                        nc.vector.tensor_add(td[:, cl], ck(6), ck(7))
                        nc.vector.tensor_add(ta[:, cl], ta[:, cl], tb[:, cl])
                        nc.vector.tensor_add(tc_[:, cl], tc_[:, cl], td[:, cl])
                        nc.vector.tensor_add(cs_h[:, cl], ta[:, cl], tc_[:, cl])
                    else:
                        nc.vector.tensor_add(ta[:, cl], ck(0), ck(1))
                        nc.vector.tensor_add(tb[:, cl], ck(4), ck(5))
                        nc.vector.tensor_add(ta[:, cl], ta[:, cl], ck(2))
                        nc.vector.tensor_add(tb[:, cl], tb[:, cl], ck(6))
                        nc.vector.tensor_add(ta[:, cl], ta[:, cl], ck(3))
                        nc.vector.tensor_add(tb[:, cl], tb[:, cl], ck(7))
                        nc.vector.tensor_add(cs_h[:, cl], ta[:, cl], tb[:, cl])

                while len(pend_stores) >= STORE_DELAY:
                    flush_store()

                out_sb = opool.tile([128, TLt], F32, tag="out_sb")

                if SKIP_COMPUTE:
                    nc.vector.tensor_copy(out_sb[:], sig_h[:])
                    pend_stores.append((out_v, out_sb))
                    continue

                for s in range(NSUB):
                    cols = slice(s * SUB * 128, (s + 1) * SUB * 128)

                    # ---- transpose signal + csum blocks into feature-major ----
                    if PS_IN_SHARED:
                        # both into ONE bf16 psum bank so B_IN=2
                        # double-buffers across subtiles
                        ps_in = psum.tile([128, 2 * SUB * 128], BF16,
                                          tag="ps_in", bufs=B_IN)
                        ps_sig = ps_in[:, :SUB * 128]
                        ps_cs = ps_in[:, SUB * 128:]
                        for j in range(SUB):
                            mb = (s * SUB + j) * 128
                            nc.tensor.transpose(
                                ps_sig[:, j * 128:(j + 1) * 128],
                                sig_h[:, mb:mb + 128], ident)
                            nc.tensor.transpose(
                                ps_cs[:, j * 128:(j + 1) * 128],
                                cs_h[:, mb:mb + 128], ident)
                    else:
                        ps_sig = psum.tile([128, SUB * 128], BF16,
                                           tag="ps_in", bufs=B_IN)
                        for j in range(SUB):
                            mb = (s * SUB + j) * 128
                            nc.tensor.transpose(
                                ps_sig[:, j * 128:(j + 1) * 128],
                                sig_h[:, mb:mb + 128], ident)
                        ps_cs = psum.tile([128, SUB * 128], BF16,
                                          tag="ps_in", bufs=B_IN)
                        for j in range(SUB):
                            mb = (s * SUB + j) * 128
                            nc.tensor.transpose(
                                ps_cs[:, j * 128:(j + 1) * 128],
                                cs_h[:, mb:mb + 128], ident)
                    sigT = acts.tile([128, SUB * 128], BF16, tag="sigT")
                    nc.scalar.activation(sigT[:], ps_sig[:], ACT.Copy)
                    csT = acts.tile([128, SUB * 128], BF16, tag="csT")
                    nc.vector.tensor_copy(csT[:], ps_cs[:])

                    # ---- L0: h0 = relu(A@sigT + Cm@csT + b0') ----
                    ps_h0a = psum.tile([128, SUB * 128], F32, tag="ha",
                                       bufs=B_HA)
                    nc.tensor.matmul(ps_h0a[:], wh(_H_W0A_SIG),
                                     sigT[:], start=True, stop=False)
                    nc.tensor.matmul(ps_h0a[:], wh(_H_W0A_CS),
                                     csT[:], start=False, stop=True)
                    ps_h0b = psum.tile([4, SUB * 128], F32, tag="hb", bufs=B_HB)
                    nc.tensor.matmul(ps_h0b[:], wh(_H_W0B_SIG, 4),
                                     sigT[:], start=True, stop=False)
                    nc.tensor.matmul(ps_h0b[:], wh(_H_W0B_CS, 4),
                                     csT[:], start=False, stop=True)
                    h0a = acts.tile([128, SUB * 128], BF16, tag="h0a")
                    nc.vector.tensor_scalar(h0a[:], ps_h0a[:],
                                            wf(_F_B0A), 0.0, ADD, MAX)
                    h0b = acts.tile([4, SUB * 128], BF16, tag="h0b")
                    nc.scalar.activation(h0b[:], ps_h0b[:], ACT.Relu,
                                         bias=wf(_F_B0B, parts=4))

                    # ---- L1 ----
                    ps_h1a = psum.tile([128, SUB * 128], F32, tag="ha",
                                       bufs=B_HA)
                    nc.tensor.matmul(ps_h1a[:], wh(_H_W1A_HI),
                                     h0a[:], start=True, stop=False)
                    nc.tensor.matmul(ps_h1a[:], wh(_H_W1A_LO, 128, parts=4),
                                     h0b[:], start=False, stop=True)
                    ps_h1b = psum.tile([4, SUB * 128], F32, tag="hb", bufs=B_HB)
                    nc.tensor.matmul(ps_h1b[:], wh(_H_W1B_HI, 4),
                                     h0a[:], start=True, stop=False)
                    nc.tensor.matmul(ps_h1b[:], wh(_H_W1B_LO, 4, parts=4),
                                     h0b[:], start=False, stop=True)
                    h1a = acts.tile([128, SUB * 128], BF16, tag="h1a")
                    nc.vector.tensor_scalar(h1a[:], ps_h1a[:],
                                            wf(_F_B1A), 0.0, ADD, MAX)
                    h1b = acts.tile([4, SUB * 128], BF16, tag="h1b")
                    nc.scalar.activation(h1b[:], ps_h1b[:], ACT.Relu,
                                         bias=wf(_F_B1B, parts=4))

                    # ---- L2 ----
                    ps_h2a = psum.tile([128, SUB * 128], F32, tag="ha",
                                       bufs=B_HA)
                    nc.tensor.matmul(ps_h2a[:], wh(_H_W2A_HI),
                                     h1a[:], start=True, stop=False)
                    nc.tensor.matmul(ps_h2a[:], wh(_H_W2A_LO, 128, parts=4),
                                     h1b[:], start=False, stop=True)
                    ps_h2b = psum.tile([4, SUB * 128], F32, tag="hb", bufs=B_HB)
                    nc.tensor.matmul(ps_h2b[:], wh(_H_W2B_HI, 4),
                                     h1a[:], start=True, stop=False)
                    nc.tensor.matmul(ps_h2b[:], wh(_H_W2B_LO, 4, parts=4),
                                     h1b[:], start=False, stop=True)
                    h2a = acts.tile([128, SUB * 128], BF16, tag="h2a")
                    nc.scalar.activation(h2a[:], ps_h2a[:], ACT.Relu,
                                         bias=wf(_F_B2A))
                    h2b = acts.tile([4, SUB * 128], BF16, tag="h2b")
                    nc.scalar.activation(h2b[:], ps_h2b[:], ACT.Relu,
                                         bias=wf(_F_B2B, parts=4))

                    # ---- L3: outT = W3 @ h2 + b3 (feature-major) ----
                    ps_oT = psum.tile([128, SUB * 128], F32, tag="po",
                                      bufs=B_PO)
                    nc.tensor.matmul(ps_oT[:], wh(_H_W3_HI),
                                     h2a[:], start=True, stop=False)
                    nc.tensor.matmul(ps_oT[:], wh(_H_W3_LO, 128, parts=4),
                                     h2b[:], start=False, stop=True)
                    oT = acts.tile([128, SUB * 128], BF16, tag="oT")
                    nc.scalar.activation(oT[:], ps_oT[:], ACT.Identity,
                                         bias=wf(_F_B3))

                    # ---- transpose back to row-major and stage the store ----
                    ps_on = psum.tile([128, SUB * 128], BF16, tag="po2",
                                      bufs=B_PO2)
                    for j in range(SUB):
                        nc.tensor.transpose(ps_on[:, j * 128:(j + 1) * 128],
                                            oT[:, j * 128:(j + 1) * 128], ident)
                    nc.vector.tensor_copy(out_sb[:, cols], ps_on[:])

                pend_stores.append((out_v, out_sb))

            while pend_stores:
                flush_store()

    return nc


_CACHED_NC = None


def _get_nc():
    global _CACHED_NC
    if _CACHED_NC is None:
        nc = bacc.Bacc("TRN2", target_bir_lowering=False, debug=False,
                       enable_asserts=False, num_devices=NCORES)
        _trace_kernel(nc)
        nc.compile()
        _CACHED_NC = nc
    return _CACHED_NC


def _make_in_maps(inputs):
    signal = np.ascontiguousarray(np.asarray(inputs["signal"], np.float32))
    components = np.ascontiguousarray(np.asarray(inputs["components"],
                                                 np.float32))
    wh, wfp = _build_wpacks(*[np.asarray(inputs[k], np.float32) for k in
                              ("Wm", "bm", "Wu", "bu", "W0", "b0",
                               "W1", "b1", "W2", "b2", "W3", "b3")])
    in_maps = []
    for c in range(NCORES):
        r0 = c * RB
        in_maps.append({
            "sig": signal[r0:r0 + RB],
            "comp": np.ascontiguousarray(components[:, r0:r0 + RB, :]),
            "wpackh": wh,
            "wpackf": wfp,
        })
    return in_maps


def kernel(**inputs):
    nc = _get_nc()
    res = bass_utils.run_bass_kernel_spmd(nc, _make_in_maps(inputs),
                                          core_ids=list(range(NCORES)))
    return np.concatenate([res.results[c]["out"] for c in range(NCORES)],
                          axis=0)
